# revision 2
# baseline (speedup 1.0000x reference)
"""Trainium2 Bass kernel v2 for nn_MemTransformerLM (DPFP linear-attention).

Differences vs v1 baseline:
- hT host-swizzled to position-contiguous columns, loaded in 4 column-groups
  (one DMA per kd-chunk per group) so projections start after ~1/8 of the load
  and pipeline with it.
- Projection -> relu features -> DPFP products -> feature transposes fused per
  6-position group; transposes run eagerly on the PE (DMA-xbar transposes are
  ~150GB/s serialized — measured v2 regression), one wide PSUM->SBUF copy per
  position split across Scalar/Vector.
- Relu features kept in two alignment frames (f2e even, f2o odd) so all DPFP
  product tensor_muls hit the DVE 2x bf16 mode.
- Attention consumes pre-transposed qfX/kfX tiles; per-chunk PE work is only
  scores / state / KV-update / output-transpose matmuls.
- o-projection packs rank pairs to contract K=128 (24 matmuls per head instead
  of 48 at K=64); head-0 part overlaps head-1's AllToAll.
- gamma/beta pre-broadcast on host (no PE broadcast matmuls at startup).
"""
import os
import sys
import types
from contextlib import ExitStack

for _p in ("/opt/trn_rl_repo",):
    if _p not in sys.path:
        sys.path.insert(0, _p)

import numpy as np
import ml_dtypes

import concourse.bass as bass
import concourse.mybir as mybir
import concourse.tile as tile
from concourse import bacc
from concourse.bass_utils import run_bass_kernel_spmd

BF16 = ml_dtypes.bfloat16
F32 = np.float32

SEQ, BATCH, D = 1536, 2, 1024
NH, DH, NR = 16, 64, 3
SCALE = 1.0 / float(np.sqrt(DH))
S_FOLD = float(np.sqrt(SCALE))          # folded into Wq rows (squared by DPFP)
EPS_D, EPS_LN = 1e-5, 1e-5
N_CORES = 8
HPC = NH // N_CORES                     # 2 heads per core
ROWS = SEQ * BATCH                      # 3072 batch-major rows
RPC = ROWS // N_CORES                   # 384 output rows per core
NCHUNK = ROWS // 128                    # 24 chunks of 128 rows
NCB = NCHUNK // BATCH                   # 12 chunks per batch
FEAT = 2 * DH * NR                      # 384 DPFP features per head
NKD = D // 128                          # 8 contraction chunks over d_model
PW = 3 * HPC * DH                       # 384 projection width (q|k|v)
NG, GP = 4, 6                           # 4 groups x 6 positions
FW = 132                                # padded per-block relu-feature width

dt = mybir.dt


def _install_profshim():
    try:
        import antenv
    except ImportError:
        return
    if "antenv.axon_hooks" in sys.modules:
        return
    mod = types.ModuleType("antenv.axon_hooks")
    mod._hook = None
    mod.set_axon_ntff_profile_hook = lambda h: setattr(mod, "_hook", h)
    mod.get_axon_ntff_profile_hook = lambda: mod._hook
    sys.modules["antenv.axon_hooks"] = mod
    antenv.axon_hooks = mod
    try:
        from trn_agent_boot.trn_boot import _ntff_profile_via_ctypes
        mod.set_axon_ntff_profile_hook(
            _ntff_profile_via_ctypes("/opt/axon/libaxon_pjrt.so"))
    except Exception:
        pass


def build_program():
    nc = bacc.Bacc("TRN2", target_bir_lowering=False, debug=False,
                   num_devices=N_CORES)

    hT_d = nc.declare_dram_parameter("hT", [D, ROWS], dt.bfloat16, isOutput=False)
    wall_d = nc.declare_dram_parameter("wallT", [D, PW], dt.bfloat16, isOutput=False)
    wo0_d = nc.declare_dram_parameter("woPk0", [128, 4 * D], dt.bfloat16, isOutput=False)
    wo1_d = nc.declare_dram_parameter("woPk1", [128, 4 * D], dt.bfloat16, isOutput=False)
    hs_d = nc.declare_dram_parameter("h_slice", [RPC, D], dt.float32, isOutput=False)
    mask_d = nc.declare_dram_parameter("mask2", [128, 256], dt.bfloat16, isOutput=False)
    ident_d = nc.declare_dram_parameter("ident", [128, 128], dt.bfloat16, isOutput=False)
    gam_d = nc.declare_dram_parameter("gamB", [128, D], dt.bfloat16, isOutput=False)
    bet_d = nc.declare_dram_parameter("betB", [128, D], dt.bfloat16, isOutput=False)
    out_d = nc.declare_dram_parameter("out", [RPC, D], dt.float32, isOutput=True)

    a2a_in = [nc.dram_tensor(f"a2a_in{h}", [N_CORES, DH, RPC], dt.bfloat16)
              for h in range(HPC)]
    a2a_out = [nc.dram_tensor(f"a2a_out{h}", [N_CORES, DH, RPC], dt.bfloat16)
               for h in range(HPC)]

    with tile.TileContext(nc) as tc:
        with (
            tc.tile_pool(name="const", bufs=1) as Pc,
            tc.tile_pool(name="big", bufs=1) as Pb,
            tc.tile_pool(name="work", bufs=2) as Pw,
            ExitStack() as _stack,
        ):
            _inner = ExitStack()
            # separate proj-psum and transpose-psum pools so projections of
            # group g+1 don't serialize behind transpose copies of group g
            Pp1 = _inner.enter_context(tc.tile_pool(name="ps_proj", bufs=2,
                                                    space="PSUM"))
            Ppx = _inner.enter_context(tc.tile_pool(name="ps_xp", bufs=2,
                                                    space="PSUM"))
            Ppsc = _inner.enter_context(tc.tile_pool(name="ps_scat", bufs=2,
                                                     space="PSUM"))
            Ppu = _inner.enter_context(tc.tile_pool(name="ps_u", bufs=1,
                                                    space="PSUM"))
            Pacc = _inner.enter_context(tc.tile_pool(name="ps_kv", bufs=1,
                                                     space="PSUM"))
            Ppat = Ppsc
            Pht = _inner.enter_context(tc.tile_pool(name="ht", bufs=2))
            Pws = _inner.enter_context(tc.tile_pool(name="wsb", bufs=1))
            Pf2 = _inner.enter_context(tc.tile_pool(name="f2", bufs=2))
            Pqf = _inner.enter_context(tc.tile_pool(name="qf", bufs=2))

            # ---------- constants ----------
            mask2 = Pc.tile([128, 256], dt.bfloat16, tag="mask2")
            ident = Pc.tile([128, 128], dt.bfloat16, tag="ident")
            gamB = Pc.tile([128, D], dt.bfloat16, tag="gamB")
            betB = Pc.tile([128, D], dt.bfloat16, tag="betB")
            eps_ln = Pc.tile([128, 1], dt.float32, tag="eps_ln")
            nc.vector.memset(eps_ln[:, :], EPS_LN)
            nc.sync.dma_start(mask2[:, :], mask_d[:, :])
            nc.sync.dma_start(ident[:, :], ident_d[:, :])

            # ---------- persistent big buffers ----------
            # va_all[p, pos*130 + h*65 + d]; ones column at d=64
            va_all = Pb.tile([128, NCHUNK * 130], dt.bfloat16, tag="va")
            # kf row-layout products: [p, pos*768 + h*384 + t*128 + f]
            kf_row = Pb.tile([128, NCHUNK * 2 * FEAT], dt.bfloat16, tag="kf_row")
            # transposed features: [f, pos*768 + h*384 + t*128 + i]
            qfX = Pb.tile([128, NCHUNK * 2 * FEAT], dt.bfloat16, tag="qfX")
            kfX = Pb.tile([128, NCHUNK * 2 * FEAT], dt.bfloat16, tag="kfX")
            # attention output [h*64+d, b*1536 + cl*128 + i]
            attn_buf = Pb.tile([128, ROWS], dt.bfloat16, tag="attn_buf")

            va4 = va_all[:, :].rearrange("p (c h d) -> p c h d", h=2, d=65)
            nc.vector.memset(va4[:, :, :, 64:65], 1.0)

            # ---------- weights ----------
            w_sb = Pws.tile([128, NKD * PW], dt.bfloat16, tag="w_sb")
            for kd in range(NKD):
                nc.sync.dma_start(w_sb[:, bass.ts(kd, PW)], wall_d[bass.ts(kd, 128), :])

            # ---------- phase 1+2: projections, relu frames, products, transposes ----------
            for g in range(NG):
                ht_g = []
                for kd in range(NKD):
                    t_ht = Pht.tile([128, GP * 128], dt.bfloat16, tag=f"ht{kd}",
                                    name=f"ht{kd}_{g}")
                    # hT is host-swizzled: columns already position-contiguous
                    nc.sync.dma_start(t_ht[:, :],
                                      hT_d[bass.ts(kd, 128),
                                           g * GP * 128:(g + 1) * GP * 128])
                    ht_g.append(t_ht)

                f2e = Pf2.tile([128, GP * 4 * FW], dt.bfloat16, tag="f2e")
                qf_g = Pqf.tile([128, GP * 2 * FEAT], dt.bfloat16, tag="qf_g")

                for pp in range(GP):
                    pos = g * GP + pp
                    pps = Pp1.tile([128, PW], dt.float32, tag="pps")
                    col0 = pp * 128
                    for kd in range(NKD):
                        nc.tensor.matmul(pps[:, :], ht_g[kd][:, col0:col0 + 128],
                                         w_sb[:, bass.ts(kd, PW)],
                                         start=(kd == 0), stop=(kd == NKD - 1))
                    # relu frame: blk = tensor*2+h over pps[:, 0:256]
                    pq = pps[:, 0:256].rearrange("p (b s) -> p b s", b=4, s=64)
                    f2e_p = f2e[:, pp * 4 * FW:(pp + 1) * 4 * FW].rearrange(
                        "p (b s) -> p b s", b=4, s=FW)
                    RELU = mybir.ActivationFunctionType.Relu
                    nc.scalar.activation(f2e_p[:, :, 4:68], pq[:, :, :], RELU)
                    nc.scalar.activation(f2e_p[:, :, 68:132], pq[:, :, :], RELU, scale=-1.0)
                    vac = va_all[:, pos * 130:(pos + 1) * 130].rearrange(
                        "p (h d) -> p h d", h=2, d=65)
                    nc.scalar.copy(vac[:, :, 0:64],
                                   pps[:, 256:384].rearrange("p (h d) -> p h d", h=2, d=64))

                # DPFP products in 3-position halves (finer pipelining):
                # dst[j>=t] = x[j]*x[j-t]; dst[j<t] = x[j]*x[128-t+j] (wrap)
                f2e_v = f2e[:, :].rearrange("p (q s) -> p q s", s=4 * FW)
                qf_v = qf_g[:, :].rearrange("p (q s) -> p q s", s=2 * FEAT)
                kf_v = kf_row[:, :].rearrange("p (c s) -> p c s", s=2 * FEAT)
                for hp in range(2):                     # half-groups of 3 pos
                    hsl = slice(hp * 3, hp * 3 + 3)
                    gsl = slice(g * GP + hp * 3, g * GP + hp * 3 + 3)
                    for h in range(HPC):
                        for tens in range(2):           # 0=q, 1=k
                            blk = tens * 2 + h
                            for t in (1, 2, 3):
                                x0 = blk * FW + 4
                                off = h * FEAT + (t - 1) * 128
                                if tens == 0:
                                    dst = qf_v[:, hsl, off:off + 128]
                                else:
                                    dst = kf_v[:, gsl, off:off + 128]
                                nc.vector.tensor_mul(
                                    dst[:, :, t:128],
                                    f2e_v[:, hsl, x0 + t:x0 + 128],
                                    f2e_v[:, hsl, x0:x0 + 128 - t])
                                nc.vector.tensor_mul(
                                    dst[:, :, 0:t],
                                    f2e_v[:, hsl, x0:x0 + t],
                                    f2e_v[:, hsl, x0 + 128 - t:x0 + 128])

                # feature transposes on the PE (psum bf16), wide copies out
                for pp in range(GP):
                    pos = g * GP + pp
                    qx_ps = Ppx.tile([128, 768], dt.bfloat16, tag="xp",
                                     name=f"qx{pos}")
                    for m in range(6):
                        nc.tensor.transpose(
                            qx_ps[:, bass.ts(m, 128)],
                            qf_g[:, pp * 768 + m * 128:pp * 768 + (m + 1) * 128],
                            ident[:, :])
                    nc.scalar.copy(qfX[:, pos * 768:(pos + 1) * 768], qx_ps[:, :])
                    kx_ps = Ppx.tile([128, 768], dt.bfloat16, tag="xp",
                                     name=f"kx{pos}")
                    for m in range(6):
                        nc.tensor.transpose(
                            kx_ps[:, bass.ts(m, 128)],
                            kf_row[:, pos * 768 + m * 128:pos * 768 + (m + 1) * 128],
                            ident[:, :])
                    nc.vector.tensor_copy(kfX[:, pos * 768:(pos + 1) * 768], kx_ps[:, :])

            # ---------- phase 3: attention, head-outer; A2A per head ----------
            for h in range(HPC):
                kv_acc = Pacc.tile([128, 390], dt.float32, tag="kvp", name=f"kvp{h}")
                kv_sb = None
                for cl in range(NCB):
                    sc_ps = Ppsc.tile([128, 256], dt.float32, tag="sc")
                    for b in range(BATCH):
                        base = (cl * 2 + b) * 768 + h * FEAT
                        for t in range(NR):
                            nc.tensor.matmul(sc_ps[:, bass.ts(b, 128)],
                                             kfX[:, base + t * 128:base + (t + 1) * 128],
                                             qfX[:, base + t * 128:base + (t + 1) * 128],
                                             start=(t == 0), stop=(t == NR - 1))
                    probT = Pw.tile([128, 256], dt.bfloat16, tag="probT")
                    nc.vector.tensor_mul(probT[:, :], sc_ps[:, :], mask2[:, :])

                    u_ps = Ppu.tile([128, 130], dt.float32, tag="u")
                    for b in range(BATCH):
                        pos = cl * 2 + b
                        base = pos * 768 + h * FEAT
                        va_c = va_all[:, pos * 130 + h * 65:pos * 130 + (h + 1) * 65]
                        nc.tensor.matmul(u_ps[:, bass.ts(b, 65)], probT[:, bass.ts(b, 128)],
                                         va_c, start=True, stop=(cl == 0))
                        if cl > 0:
                            for t in range(NR):
                                nc.tensor.matmul(
                                    u_ps[:, bass.ts(b, 65)],
                                    qfX[:, base + t * 128:base + (t + 1) * 128],
                                    kv_sb[:, b * 195 + t * 65:b * 195 + (t + 1) * 65],
                                    start=False, stop=(t == NR - 1))

                    # KV state update in PSUM accumulator
                    kv_pk = Pw.tile([128, 390], dt.bfloat16, tag="kv_pk")
                    for b in range(BATCH):
                        pos = cl * 2 + b
                        base = pos * 768 + h * FEAT
                        va_c = va_all[:, pos * 130 + h * 65:pos * 130 + (h + 1) * 65]
                        for t in range(NR):
                            nc.tensor.matmul(
                                kv_acc[:, b * 195 + t * 65:b * 195 + (t + 1) * 65],
                                kf_row[:, base + t * 128:base + (t + 1) * 128],
                                va_c,
                                start=(cl == 0 and b == 0 and t == 0),
                                stop=(cl == NCB - 1),
                                skip_group_check=True)
                    if cl < NCB - 1:
                        nc.scalar.copy(kv_pk[:, 0:195], kv_acc[:, 0:195])
                        nc.vector.tensor_copy(kv_pk[:, 195:390], kv_acc[:, 195:390])
                    kv_sb = kv_pk

                    # normalize: attn = u[:, :64] / (u[:, 64] + eps)
                    d2 = Pw.tile([128, 2], dt.float32, tag="d2")
                    r2 = Pw.tile([128, 2], dt.float32, tag="r2")
                    u_dn = u_ps[:, 0:130].rearrange("p (q d) -> p q d", q=2, d=65)
                    nc.vector.tensor_scalar_add(d2[:, :], u_dn[:, :, 64], EPS_D)
                    nc.vector.reciprocal(r2[:, :], d2[:, :])
                    attn2 = Pw.tile([128, 128], dt.bfloat16, tag="attn2")
                    for b in range(BATCH):
                        nc.vector.tensor_scalar_mul(attn2[:, bass.ts(b, 64)],
                                                    u_ps[:, b * 65:b * 65 + 64],
                                                    r2[:, b:b + 1])
                    at_ps = Ppat.tile([128, 256], dt.bfloat16, tag="sc")
                    for b in range(BATCH):
                        nc.tensor.transpose(at_ps[0:64, bass.ts(b, 128)],
                                            attn2[:, bass.ts(b, 64)], ident[:, :])
                    src = at_ps[0:64, 0:256].rearrange("p (b i) -> p b i", b=2, i=128)
                    dstv = attn_buf[h * 64:(h + 1) * 64, :].rearrange(
                        "p (b s) -> p b s", b=2, s=SEQ)[:, :, cl * 128:(cl + 1) * 128]
                    if h == 0:
                        nc.scalar.copy(dstv, src)
                    else:
                        nc.vector.tensor_copy(dstv, src)

                for r in range(N_CORES):
                    nc.sync.dma_start(a2a_in[h][r, :, :],
                                      attn_buf[h * 64:(h + 1) * 64, bass.ts(r, RPC)])
                nc.gpsimd.collective_compute(
                    "AllToAll", mybir.AluOpType.bypass,
                    replica_groups=[list(range(N_CORES))],
                    ins=[a2a_in[h].ap().opt()], outs=[a2a_out[h].ap().opt()])

            _inner.close()   # frees ht / w_sb / f2 / qf space + psum banks

            # ---------- phase 4: o-projection (rank-pair packed) + LN ----------
            Po = _stack.enter_context(tc.tile_pool(name="post", bufs=1))
            Pp4 = _stack.enter_context(tc.tile_pool(name="ps4", bufs=6, space="PSUM"))
            # weight/const loads on the sync rail, issued before any collective
            # waits; a2a_out gathers on the scalar rail so they don't block it
            nc.sync.dma_start(gamB[:, :], gam_d[:, :])
            nc.sync.dma_start(betB[:, :], bet_d[:, :])
            wo_d = [wo0_d, wo1_d]
            asl, wo_sb, hs_sb = [], [], []
            for h in range(HPC):
                w_t = Po.tile([128, 4 * D], dt.bfloat16, tag=f"wo{h}", name=f"wo{h}")
                nc.sync.dma_start(w_t[:, :], wo_d[h][:, :])
                wo_sb.append(w_t)
            for rc in range(3):
                h_t = Po.tile([128, D], dt.float32, tag=f"hs{rc}", name=f"hs{rc}")
                nc.sync.dma_start(h_t[:, :], hs_d[bass.ts(rc, 128), :])
                hs_sb.append(h_t)
            for h in range(HPC):
                a_t = Po.tile([128, 4 * RPC], dt.bfloat16, tag=f"asl{h}", name=f"asl{h}")
                for r in range(N_CORES):
                    nc.scalar.dma_start(
                        a_t[(r % 2) * 64:(r % 2) * 64 + 64,
                            (r // 2) * RPC:(r // 2 + 1) * RPC],
                        a2a_out[h][r, :, :])
                asl.append(a_t)

            # all head-0 matmuls first (overlap head-1's AllToAll), 6 open groups
            ops_t = [Pp4.tile([128, 512], dt.float32, tag="ops", name=f"ops{i}")
                     for i in range(6)]
            for h in range(HPC):
                for rc in range(3):
                    for n in range(2):
                        ops = ops_t[rc * 2 + n]
                        for p in range(4):
                            nc.tensor.matmul(
                                ops[:, :],
                                asl[h][:, p * RPC + rc * 128:p * RPC + (rc + 1) * 128],
                                wo_sb[h][:, p * D + n * 512:p * D + (n + 1) * 512],
                                start=(h == 0 and p == 0),
                                stop=(h == HPC - 1 and p == 3),
                                skip_group_check=True)

            for rc in range(3):
                x = Po.tile([128, D], dt.float32, tag="x", bufs=2)
                s2 = Pw.tile([128, 2], dt.float32, tag="s2")
                for n in range(2):
                    nc.vector.scalar_tensor_tensor(
                        x[:, bass.ts(n, 512)], ops_t[rc * 2 + n][:, :], 0.0,
                        hs_sb[rc][:, bass.ts(n, 512)],
                        op0=mybir.AluOpType.add, op1=mybir.AluOpType.add,
                        accum_out=s2[:, n:n + 1])
                mean = Pw.tile([128, 1], dt.float32, tag="mean")
                nc.vector.tensor_reduce(mean[:, :], s2[:, :],
                                        axis=mybir.AxisListType.X,
                                        op=mybir.AluOpType.add)
                nc.vector.tensor_scalar_mul(mean[:, :], mean[:, :], 1.0 / D)
                var = Pw.tile([128, 1], dt.float32, tag="var")
                nc.vector.tensor_scalar(x[:, :], x[:, :], mean[:, :], None,
                                        op0=mybir.AluOpType.subtract)
                sq = Po.tile([128, D], dt.float32, tag="sq", bufs=2)
                nc.gpsimd.tensor_mul(sq[:, :], x[:, :], x[:, :])
                nc.vector.tensor_reduce(var[:, :], sq[:, :],
                                        axis=mybir.AxisListType.X,
                                        op=mybir.AluOpType.add)
                rstd = Pw.tile([128, 1], dt.float32, tag="rstd")
                nc.scalar.activation(rstd[:, :], var[:, :],
                                     mybir.ActivationFunctionType.Sqrt,
                                     bias=eps_ln[:, :], scale=1.0 / D)
                nc.vector.reciprocal(rstd[:, :], rstd[:, :])
                nc.vector.scalar_tensor_tensor(
                    sq[:, :], x[:, :], rstd[:, :], gamB[:, :],
                    op0=mybir.AluOpType.mult, op1=mybir.AluOpType.mult)
                nc.gpsimd.tensor_add(sq[:, :], sq[:, :], betB[:, :])
                nc.sync.dma_start(out_d[bass.ts(rc, 128), :], sq[:, :])

    nc.finalize()
    return nc


_PROGRAM = None


def _get_program():
    global _PROGRAM
    if _PROGRAM is None:
        _PROGRAM = build_program()
    return _PROGRAM


def _host_prep(h, Wq, Wkv, Wo, ln_gamma, ln_beta):
    h = np.asarray(h, F32)
    h_bm = np.ascontiguousarray(h.transpose(1, 0, 2).reshape(ROWS, D))
    # position-swizzled hT: column pos*128+i <- row (pos%2)*1536+(pos//2)*128+i
    h_sw = h_bm.reshape(BATCH, NCB, 128, D).transpose(1, 0, 2, 3).reshape(ROWS, D)
    hT = np.ascontiguousarray(h_sw.T).astype(BF16)
    Wq_h = np.asarray(Wq, F32).reshape(NH, DH, D)
    Wk_h = np.asarray(Wkv, F32)[:NH * DH].reshape(NH, DH, D)
    Wv_h = np.asarray(Wkv, F32)[NH * DH:].reshape(NH, DH, D)
    WoT = np.ascontiguousarray(np.asarray(Wo, F32).T)          # [hd, D]
    # rank-pair packed Wo: woPk_h[s*64+d, p*D+col] = WoT[(2p+s)*128 + h*64 + d, col]
    WoT5 = WoT.reshape(4, 2, 2, 64, D)                         # [p, s, hh, d, col]
    woPk = [np.ascontiguousarray(
        WoT5[:, :, hh].transpose(1, 2, 0, 3).reshape(128, 4 * D)).astype(BF16)
        for hh in range(HPC)]
    mask2 = np.tile(np.triu(np.ones((128, 128), F32)), (1, 2)).astype(BF16)
    ident = np.eye(128, dtype=F32).astype(BF16)
    gamB = np.ascontiguousarray(
        np.broadcast_to(np.asarray(ln_gamma, F32).reshape(1, D), (128, D))).astype(BF16)
    betB = np.ascontiguousarray(
        np.broadcast_to(np.asarray(ln_beta, F32).reshape(1, D), (128, D))).astype(BF16)

    in_maps = []
    for core in range(N_CORES):
        hh = [HPC * core + i for i in range(HPC)]
        W_all = np.concatenate([
            np.concatenate([Wq_h[j] * S_FOLD for j in hh]),
            np.concatenate([Wk_h[j] for j in hh]),
            np.concatenate([Wv_h[j] for j in hh]),
        ])
        in_maps.append({
            "hT": hT,
            "wallT": np.ascontiguousarray(W_all.T).astype(BF16),
            "woPk0": woPk[0],
            "woPk1": woPk[1],
            "h_slice": np.ascontiguousarray(h_bm[core * RPC:(core + 1) * RPC]),
            "mask2": mask2,
            "ident": ident,
            "gamB": gamB,
            "betB": betB,
        })
    return in_maps


def run(inputs, trace=False):
    _install_profshim()
    nc = _get_program()
    in_maps = _host_prep(inputs["h"], inputs["Wq"], inputs["Wkv"], inputs["Wo"],
                         inputs["ln_gamma"], inputs["ln_beta"])
    res = run_bass_kernel_spmd(nc, in_maps, core_ids=list(range(N_CORES)),
                               trace=trace)
    out_bm = np.concatenate([res.results[c]["out"] for c in range(N_CORES)], axis=0)
    out = out_bm.reshape(BATCH, SEQ, D).transpose(1, 0, 2).astype(F32)
    return np.ascontiguousarray(out), res


def kernel(**inputs):
    out, _ = run(inputs, trace=False)
    return out
